# revision 30
# baseline (speedup 1.0000x reference)
"""Trainium2 Bass kernel for nn_AttentionModel (B=4, S=2048, H=8, D=64).

Sharding: 32 (batch, head) pairs split 4-per-core across 8 NeuronCores
(data + head parallel). Each core runs full attention for its 4 heads,
processed as 2 head-pairs so the D=64 contractions pack into the 128-row
PE array and the 64x64 projections become one 128x128 block-diagonal
matmul per pair.

Inputs are pre-transposed on the host to [d, s] (d-major) layout so the
kernel needs no PE transposes at all; the output is produced in [e, s]
layout and de-transposed on the host.

Per-core pipeline, per head-pair:
  prep:  qT/kT = blockdiag(W) @ xT + b        (PSUM -> DVE bias-add)
         v'[j, e|1] = [xv | 1-row] @ [[Wv^T];[bv]] aug matmul per j-tile
         (ones column makes the softmax denominator fall out of PV)
  attn (hardware For_i loop over 4 query chunks of 512):
         stage q chunk (DMA), scores^T[j, i] = k^T_jt.T @ q^T  (f32r,
         2 heads row-packed in the PE array), DVE evacuates PSUM,
         ACT Exp (scale=1/8) over [128, 4096] groups in-place,
         acc[e|den, i] += v'_jt.T @ exp  accumulated over j-tiles,
         reciprocal of den row -> gpsimd partition-broadcast -> DVE mul,
         DMA out in [e, i] layout.

Softmax skips the max-subtraction: scores are ~N(0, 0.33); exp stays well
inside f32 range so the result matches jax.nn.softmax to f32 precision.
"""
import numpy as np

B, S, H, D = 4, 2048, 8, 64
NCORES = 8
HPC = 4            # heads per core
NPAIR = 2          # head pairs per core
NJ = 16            # key tiles of 128
IC = 512           # query-chunk width
NCH = S // IC      # 4 chunks

_cache = {}


def _build(repeat=1):
    import concourse.bacc as bacc
    import concourse.mybir as mybir
    from concourse.tile import TileContext
    from concourse.bass import ts, ds

    F32 = mybir.dt.float32
    F32R = mybir.dt.float32r
    AF = mybir.ActivationFunctionType

    nc = bacc.Bacc("TRN2", target_bir_lowering=False, debug=False,
                   num_devices=NCORES)

    xqT = nc.declare_dram_parameter("xqT", [NPAIR, 128, S], F32, isOutput=False)
    xkT = nc.declare_dram_parameter("xkT", [NPAIR, 128, S], F32, isOutput=False)
    xvA = nc.declare_dram_parameter("xvA", [NPAIR, 2, 65, S], F32, isOutput=False)
    wq2 = nc.declare_dram_parameter("wq2", [128, 128], F32, isOutput=False)
    wk2 = nc.declare_dram_parameter("wk2", [128, 128], F32, isOutput=False)
    wva = nc.declare_dram_parameter("wva", [65, 66], F32, isOutput=False)
    bq2 = nc.declare_dram_parameter("bq2", [128, 1], F32, isOutput=False)
    bk2 = nc.declare_dram_parameter("bk2", [128, 1], F32, isOutput=False)
    out_dr = nc.declare_dram_parameter("out", [HPC, D, S], F32, isOutput=True)

    with TileContext(nc) as tc:
        with (
            tc.tile_pool(name="constp", bufs=1) as constp,
            tc.tile_pool(name="xldp", bufs=1) as xldp,
            tc.tile_pool(name="augp", bufs=1) as augp,
            tc.tile_pool(name="qkvp", bufs=1) as qkvp,
            tc.tile_pool(name="vpp", bufs=1) as vpp,
            tc.tile_pool(name="scp", bufs=1) as scp,
            tc.tile_pool(name="qchp", bufs=2) as qchp,
            tc.tile_pool(name="obp", bufs=2) as obp,
            tc.tile_pool(name="psmm", bufs=3, space="PSUM") as psmm,
            tc.tile_pool(name="psacc", bufs=1, space="PSUM") as psacc,
        ):
            wq_sb = constp.tile([128, 128], F32R, name="wq_sb", tag="wq")
            nc.sync.dma_start(wq_sb[:], wq2[:, :].bitcast(F32R))
            wk_sb = constp.tile([128, 128], F32R, name="wk_sb", tag="wk")
            nc.sync.dma_start(wk_sb[:], wk2[:, :].bitcast(F32R))
            wva_sb = constp.tile([65, 66], F32R, name="wva_sb", tag="wva")
            nc.sync.dma_start(wva_sb[:], wva[:, :].bitcast(F32R))
            bq_sb = constp.tile([128, 1], F32, name="bq_sb", tag="bq")
            nc.sync.dma_start(bq_sb[:], bq2[:, :])
            bk_sb = constp.tile([128, 1], F32, name="bk_sb", tag="bk")
            nc.sync.dma_start(bk_sb[:], bk2[:, :])

            for rep in range(repeat):
                # Splice the staggered loop's entry barrier BEFORE the prep
                # work: engines enter the chunk loop as their own prep deps
                # clear instead of waiting for all-engine prep completion.
                tc.prologue_barrier()
                qT2s, kT2s, vp_sbs = [], [], []
                for p in range(NPAIR):
                    xq_sb = xldp.tile([128, S], F32R, name=f"xq_{p}_{rep}",
                                      tag="xq")
                    xk_sb = xldp.tile([128, S], F32R, name=f"xk_{p}_{rep}",
                                      tag="xk")
                    for hv in range(2):
                        # halved loads: the first projection matmuls start
                        # at half-arrival instead of after the full 1 MB
                        nc.sync.dma_start(xq_sb[:, ts(hv, S // 2)],
                                          xqT[p, :, ts(hv, S // 2)].bitcast(F32R))
                        nc.sync.dma_start(xk_sb[:, ts(hv, S // 2)],
                                          xkT[p, :, ts(hv, S // 2)].bitcast(F32R))
                    aug = []
                    for h in range(2):
                        a = augp.tile([65, S], F32R, name=f"aug_{p}_{h}_{rep}",
                                      tag=f"aug{h}")
                        nc.sync.dma_start(a[:], xvA[p, h, :, :].bitcast(F32R))
                        aug.append(a)

                    qT2 = qkvp.tile([128, S], F32R, name=f"qT2_{p}_{rep}",
                                    tag=f"qT2_{p}")
                    kT2 = qkvp.tile([128, S], F32R, name=f"kT2_{p}_{rep}",
                                    tag=f"kT2_{p}")
                    qT2s.append(qT2)
                    kT2s.append(kT2)
                    for w_sb, b_sb, x_sb, dst in ((wq_sb, bq_sb, xq_sb, qT2),
                                                  (wk_sb, bk_sb, xk_sb, kT2)):
                        for c2 in range(NCH // 2):
                            pp = psmm.tile([128, 2 * IC], F32,
                                           name=f"pp_{p}_{dst.name}_{c2}_{rep}",
                                           tag="mm")
                            for u in range(2):
                                nc.tensor.matmul(pp[:, u * IC:(u + 1) * IC],
                                                 w_sb[:],
                                                 x_sb[:, ts(2 * c2 + u, IC)],
                                                 start=True, stop=True)
                            # ACT does the PSUM->SBUF move + bias in one
                            # op; prep-phase ACT is otherwise idle.
                            nc.scalar.activation(dst[:, ts(c2, 2 * IC)], pp[:],
                                                 AF.Identity, bias=b_sb[:, 0:1])

                    vp_sb = []
                    for h in range(2):
                        vp = vpp.tile([128, NJ * 66], F32R,
                                      name=f"vp_{p}_{h}_{rep}", tag=f"vp{p}{h}")
                        vp_sb.append(vp)
                    vp_sbs.append(vp_sb)
                    for g in range(4):
                        for h in range(2):
                            pv4 = psmm.tile([128, 264], F32,
                                            name=f"pv4_{p}_{h}_{g}_{rep}",
                                            tag="mm")
                            for u in range(4):
                                jt = g * 4 + u
                                nc.tensor.matmul(pv4[:, u * 66:(u + 1) * 66],
                                                 aug[h][:, ts(jt, 128)],
                                                 wva_sb[:],
                                                 start=True, stop=True)
                            nc.scalar.activation(
                                vp_sb[h][:, g * 264:(g + 1) * 264], pv4[:],
                                AF.Copy)

                # One loop over query chunks covering BOTH head pairs: the
                # two pairs' chains are independent, so the scheduler can
                # overlap pair 0's exp/PV tail with pair 1's scores/evac.
                # They time-share one sc2 buffer (subtile WAR chaining) and
                # ping-pong the single PV-accumulator slot.
                with tc.For_i(0, S, IC, staggered_reset=True) as ci:
                    sc2 = scp.tile([128, NJ * 1024], F32R, name=f"sc2_{rep}",
                                   tag="sc2")
                    for p in range(NPAIR):
                        qT2, kT2, vp_sb = qT2s[p], kT2s[p], vp_sbs[p]
                        qch = qchp.tile([128, IC], F32R, name=f"qch_{p}_{rep}",
                                        tag=f"qch{p}")
                        nc.sync.dma_start(qch[:], qT2[:, ds(ci, IC)])
                        for jt in range(NJ):
                            sp2 = psmm.tile([128, 1024], F32,
                                            name=f"sp2_{p}_{jt}_{rep}", tag="mm")
                            for h in range(2):
                                nc.tensor.matmul(
                                    sp2[:, h * IC:(h + 1) * IC],
                                    kT2[h * 64:h * 64 + 64, ts(jt, 128)],
                                    qch[h * 64:h * 64 + 64, :],
                                    start=True, stop=True,
                                    tile_position=(h * 64, 0),
                                )
                            if jt < 4:
                                # ACT exps straight out of PSUM: offloads the
                                # DVE evacuation for these tiles and lets PV
                                # start early. Keep these FIRST in ACT's
                                # FIFO: their inputs are ready before the
                                # batched exps' evacuations.
                                nc.scalar.activation(
                                    sc2[:, jt * 1024:(jt + 1) * 1024], sp2[:],
                                    AF.Exp, scale=0.125)
                            else:
                                nc.vector.tensor_copy(
                                    sc2[:, jt * 1024:(jt + 1) * 1024], sp2[:])
                        for g in range(1, 4):
                            nc.scalar.activation(
                                sc2[:, g * 4096:(g + 1) * 4096],
                                sc2[:, g * 4096:(g + 1) * 4096],
                                AF.Exp, scale=0.125)
                        acc = psacc.tile([65, 1024], F32, name=f"acc_{p}_{rep}",
                                         tag="acc")
                        for jt in range(NJ):
                            for h in range(2):
                                nc.tensor.matmul(
                                    acc[:, h * IC:(h + 1) * IC],
                                    vp_sb[h][:, jt * 66:jt * 66 + 65],
                                    sc2[:, jt * 1024 + h * IC:jt * 1024 + (h + 1) * IC],
                                    start=(jt == 0), stop=(jt == NJ - 1),
                                )
                        for h in range(2):
                            osb = obp.tile([65, IC], F32,
                                           name=f"osb_{p}_{h}_{rep}", tag="osb")
                            nc.vector.tensor_copy(osb[:],
                                                  acc[:, h * IC:(h + 1) * IC])
                            rec0 = obp.tile([1, IC], F32,
                                            name=f"rec_{p}_{h}_{rep}", tag="rec")
                            nc.vector.reciprocal(rec0[:], osb[64:65, :])
                            rb = obp.tile([64, IC], F32,
                                          name=f"rb_{p}_{h}_{rep}", tag="rb")
                            nc.gpsimd.partition_broadcast(rb[:], rec0[:])
                            ot = obp.tile([64, IC], F32,
                                          name=f"ot_{p}_{h}_{rep}", tag="ot")
                            nc.gpsimd.tensor_mul(ot[:], osb[0:64, :], rb[:])
                            nc.sync.dma_start(out_dr[2 * p + h, :, ds(ci, IC)],
                                              ot[:])

    nc.compile()
    return nc


def _prep_inputs(query, key, value, Wq, bq, Wk, bk, Wv, bv):
    """Host-side sharding/layout prep. Returns per-core input maps."""
    def d_major(x):
        # [B, S, H, D] -> [B*H, D, S]
        xt = np.asarray(x, np.float32).transpose(0, 2, 3, 1)
        return np.ascontiguousarray(xt.reshape(B * H, D, S))

    qh, kh, vh = d_major(query), d_major(key), d_major(value)

    def blockdiag(W):
        Wt = np.asarray(W, np.float32).T  # [d, e]
        W2 = np.zeros((128, 128), np.float32)
        W2[:64, :64] = Wt
        W2[64:, 64:] = Wt
        return W2

    wva = np.zeros((65, 66), np.float32)
    wva[:64, :64] = np.asarray(Wv, np.float32).T
    wva[64, :64] = np.asarray(bv, np.float32)
    wva[64, 64] = 1.0

    def bias2(b):
        return np.concatenate([np.asarray(b, np.float32)] * 2).reshape(128, 1)

    shared = dict(wq2=blockdiag(Wq), wk2=blockdiag(Wk), wva=wva,
                  bq2=bias2(bq), bk2=bias2(bk))
    in_maps = []
    for c in range(NCORES):
        sl = slice(c * HPC, (c + 1) * HPC)
        va = vh[sl].reshape(NPAIR, 2, 64, S)
        va = np.concatenate(
            [va, np.ones((NPAIR, 2, 1, S), np.float32)], axis=2)
        in_maps.append(dict(
            xqT=np.ascontiguousarray(qh[sl]).reshape(NPAIR, 128, S),
            xkT=np.ascontiguousarray(kh[sl]).reshape(NPAIR, 128, S),
            xvA=np.ascontiguousarray(va),
            **shared))
    return in_maps


def kernel(query, key, value, Wq, bq, Wk, bk, Wv, bv):
    from concourse.bass_utils import run_bass_kernel_spmd

    if "nc" not in _cache:
        _cache["nc"] = _build()
    nc = _cache["nc"]

    in_maps = _prep_inputs(query, key, value, Wq, bq, Wk, bk, Wv, bv)
    res = run_bass_kernel_spmd(nc, in_maps, list(range(NCORES)))
    out = np.stack([res.results[c]["out"] for c in range(NCORES)])  # [8,4,D,S]
    out = out.reshape(B, H, D, S).transpose(0, 3, 1, 2)  # [B,S,H,D]
    return np.ascontiguousarray(out)


# revision 33
# speedup vs baseline: 5.9926x; 5.9926x over previous
"""Trainium2 Bass kernel for nn_AttentionModel (B=4, S=2048, H=8, D=64).

Sharding: 32 (batch, head) pairs split 4-per-core across 8 NeuronCores
(data + head parallel). Each core runs full attention for its 4 heads,
processed as 2 head-pairs so the D=64 contractions pack into the 128-row
PE array and the 64x64 projections become one 128x128 block-diagonal
matmul per pair.

Inputs are pre-transposed on the host to [d, s] (d-major) layout so the
kernel needs no PE transposes at all; the output is produced in [e, s]
layout and de-transposed on the host.

Per-core pipeline, per head-pair:
  prep:  qT/kT = blockdiag(W) @ xT + b        (PSUM -> DVE bias-add)
         v'[j, e|1] = [xv | 1-row] @ [[Wv^T];[bv]] aug matmul per j-tile
         (ones column makes the softmax denominator fall out of PV)
  attn (hardware For_i loop over 4 query chunks of 512):
         stage q chunk (DMA), scores^T[j, i] = k^T_jt.T @ q^T  (f32r,
         2 heads row-packed in the PE array), DVE evacuates PSUM,
         ACT Exp (scale=1/8) over [128, 4096] groups in-place,
         acc[e|den, i] += v'_jt.T @ exp  accumulated over j-tiles,
         reciprocal of den row -> gpsimd partition-broadcast -> DVE mul,
         DMA out in [e, i] layout.

Softmax skips the max-subtraction: scores are ~N(0, 0.33); exp stays well
inside f32 range so the result matches jax.nn.softmax to f32 precision.
"""
import numpy as np

B, S, H, D = 4, 2048, 8, 64
NCORES = 8
HPC = 4            # heads per core
NPAIR = 2          # head pairs per core
NJ = 16            # key tiles of 128
IC = 512           # query-chunk width
NCH = S // IC      # 4 chunks

_cache = {}


def _build(repeat=1):
    import concourse.bacc as bacc
    import concourse.mybir as mybir
    from concourse.tile import TileContext
    from concourse.bass import ts, ds

    F32 = mybir.dt.float32
    F32R = mybir.dt.float32r
    AF = mybir.ActivationFunctionType

    nc = bacc.Bacc("TRN2", target_bir_lowering=False, debug=False,
                   num_devices=NCORES)

    xqT = nc.declare_dram_parameter("xqT", [NPAIR, 128, S], F32, isOutput=False)
    xkT = nc.declare_dram_parameter("xkT", [NPAIR, 128, S], F32, isOutput=False)
    xvA = nc.declare_dram_parameter("xvA", [NPAIR, 2, 65, S], F32, isOutput=False)
    wq2 = nc.declare_dram_parameter("wq2", [128, 128], F32, isOutput=False)
    wk2 = nc.declare_dram_parameter("wk2", [128, 128], F32, isOutput=False)
    wva = nc.declare_dram_parameter("wva", [65, 66], F32, isOutput=False)
    bq2 = nc.declare_dram_parameter("bq2", [128, 1], F32, isOutput=False)
    bk2 = nc.declare_dram_parameter("bk2", [128, 1], F32, isOutput=False)
    out_dr = nc.declare_dram_parameter("out", [HPC, D, S], F32, isOutput=True)

    with TileContext(nc) as tc:
        with (
            tc.tile_pool(name="constp", bufs=1) as constp,
            tc.tile_pool(name="xldp", bufs=1) as xldp,
            tc.tile_pool(name="augp", bufs=1) as augp,
            tc.tile_pool(name="qkvp", bufs=1) as qkvp,
            tc.tile_pool(name="vpp", bufs=1) as vpp,
            tc.tile_pool(name="scp", bufs=1) as scp,
            tc.tile_pool(name="qchp", bufs=2) as qchp,
            tc.tile_pool(name="obp", bufs=2) as obp,
            tc.tile_pool(name="psmm", bufs=3, space="PSUM") as psmm,
            tc.tile_pool(name="psacc", bufs=1, space="PSUM") as psacc,
        ):
            wq_sb = constp.tile([128, 128], F32R, name="wq_sb", tag="wq")
            nc.sync.dma_start(wq_sb[:], wq2[:, :].bitcast(F32R))
            wk_sb = constp.tile([128, 128], F32R, name="wk_sb", tag="wk")
            nc.sync.dma_start(wk_sb[:], wk2[:, :].bitcast(F32R))
            wva_sb = constp.tile([65, 66], F32R, name="wva_sb", tag="wva")
            nc.sync.dma_start(wva_sb[:], wva[:, :].bitcast(F32R))
            bq_sb = constp.tile([128, 1], F32, name="bq_sb", tag="bq")
            nc.sync.dma_start(bq_sb[:], bq2[:, :])
            bk_sb = constp.tile([128, 1], F32, name="bk_sb", tag="bk")
            nc.sync.dma_start(bk_sb[:], bk2[:, :])

            for rep in range(repeat):
                # Splice the staggered loop's entry barrier BEFORE the prep
                # work: engines enter the chunk loop as their own prep deps
                # clear instead of waiting for all-engine prep completion.
                tc.prologue_barrier()
                qT2s, kT2s, vp_sbs = [], [], []
                for p in range(NPAIR):
                    xq_sb = xldp.tile([128, S], F32R, name=f"xq_{p}_{rep}",
                                      tag="xq")
                    xk_sb = xldp.tile([128, S], F32R, name=f"xk_{p}_{rep}",
                                      tag="xk")
                    for hv in range(2):
                        # halved loads: the first projection matmuls start
                        # at half-arrival instead of after the full 1 MB
                        nc.sync.dma_start(xq_sb[:, ts(hv, S // 2)],
                                          xqT[p, :, ts(hv, S // 2)].bitcast(F32R))
                        nc.sync.dma_start(xk_sb[:, ts(hv, S // 2)],
                                          xkT[p, :, ts(hv, S // 2)].bitcast(F32R))
                    aug = []
                    for h in range(2):
                        a = augp.tile([65, S], F32R, name=f"aug_{p}_{h}_{rep}",
                                      tag=f"aug{h}")
                        nc.sync.dma_start(a[:], xvA[p, h, :, :].bitcast(F32R))
                        aug.append(a)

                    qT2 = qkvp.tile([128, S], F32R, name=f"qT2_{p}_{rep}",
                                    tag=f"qT2_{p}")
                    kT2 = qkvp.tile([128, S], F32R, name=f"kT2_{p}_{rep}",
                                    tag=f"kT2_{p}")
                    qT2s.append(qT2)
                    kT2s.append(kT2)
                    for w_sb, b_sb, x_sb, dst in ((wq_sb, bq_sb, xq_sb, qT2),
                                                  (wk_sb, bk_sb, xk_sb, kT2)):
                        for c2 in range(NCH // 2):
                            pp = psmm.tile([128, 2 * IC], F32,
                                           name=f"pp_{p}_{dst.name}_{c2}_{rep}",
                                           tag="mm")
                            for u in range(2):
                                nc.tensor.matmul(pp[:, u * IC:(u + 1) * IC],
                                                 w_sb[:],
                                                 x_sb[:, ts(2 * c2 + u, IC)],
                                                 start=True, stop=True)
                            # ACT does the PSUM->SBUF move + bias in one
                            # op; prep-phase ACT is otherwise idle.
                            nc.scalar.activation(dst[:, ts(c2, 2 * IC)], pp[:],
                                                 AF.Identity, bias=b_sb[:, 0:1])

                    vp_sb = []
                    for h in range(2):
                        vp = vpp.tile([128, NJ * 66], F32R,
                                      name=f"vp_{p}_{h}_{rep}", tag=f"vp{p}{h}")
                        vp_sb.append(vp)
                    vp_sbs.append(vp_sb)
                    for g in range(4):
                        for h in range(2):
                            pv4 = psmm.tile([128, 264], F32,
                                            name=f"pv4_{p}_{h}_{g}_{rep}",
                                            tag="mm")
                            for u in range(4):
                                jt = g * 4 + u
                                nc.tensor.matmul(pv4[:, u * 66:(u + 1) * 66],
                                                 aug[h][:, ts(jt, 128)],
                                                 wva_sb[:],
                                                 start=True, stop=True)
                            nc.scalar.activation(
                                vp_sb[h][:, g * 264:(g + 1) * 264], pv4[:],
                                AF.Copy)

                # One loop over query chunks covering BOTH head pairs: the
                # two pairs' chains are independent, so the scheduler can
                # overlap pair 0's exp/PV tail with pair 1's scores/evac.
                # They time-share one sc2 buffer (subtile WAR chaining) and
                # ping-pong the single PV-accumulator slot.
                with tc.For_i(0, S, IC, staggered_reset=True) as ci:
                    sc2 = scp.tile([128, NJ * 1024], F32R, name=f"sc2_{rep}",
                                   tag="sc2")
                    for p in range(NPAIR):
                        qT2, kT2, vp_sb = qT2s[p], kT2s[p], vp_sbs[p]
                        qch = qchp.tile([128, IC], F32R, name=f"qch_{p}_{rep}",
                                        tag=f"qch{p}")
                        nc.sync.dma_start(qch[:], qT2[:, ds(ci, IC)])
                        for jt in range(NJ):
                            sp2 = psmm.tile([128, 1024], F32,
                                            name=f"sp2_{p}_{jt}_{rep}", tag="mm")
                            for h in range(2):
                                nc.tensor.matmul(
                                    sp2[:, h * IC:(h + 1) * IC],
                                    kT2[h * 64:h * 64 + 64, ts(jt, 128)],
                                    qch[h * 64:h * 64 + 64, :],
                                    start=True, stop=True,
                                    tile_position=(h * 64, 0),
                                )
                            if jt < 4:
                                # ACT exps straight out of PSUM: offloads the
                                # DVE evacuation for these tiles and lets PV
                                # start early. Keep these FIRST in ACT's
                                # FIFO: their inputs are ready before the
                                # batched exps' evacuations.
                                nc.scalar.activation(
                                    sc2[:, jt * 1024:(jt + 1) * 1024], sp2[:],
                                    AF.Exp, scale=0.125)
                            else:
                                nc.vector.tensor_copy(
                                    sc2[:, jt * 1024:(jt + 1) * 1024], sp2[:])
                        for g in range(1, 4):
                            nc.scalar.activation(
                                sc2[:, g * 4096:(g + 1) * 4096],
                                sc2[:, g * 4096:(g + 1) * 4096],
                                AF.Exp, scale=0.125)
                        acc = psacc.tile([65, 1024], F32, name=f"acc_{p}_{rep}",
                                         tag="acc")
                        for jt in range(NJ):
                            for h in range(2):
                                nc.tensor.matmul(
                                    acc[:, h * IC:(h + 1) * IC],
                                    vp_sb[h][:, jt * 66:jt * 66 + 65],
                                    sc2[:, jt * 1024 + h * IC:jt * 1024 + (h + 1) * IC],
                                    start=(jt == 0), stop=(jt == NJ - 1),
                                )
                        for h in range(2):
                            osb = obp.tile([65, IC], F32,
                                           name=f"osb_{p}_{h}_{rep}", tag="osb")
                            nc.vector.tensor_copy(osb[:],
                                                  acc[:, h * IC:(h + 1) * IC])
                            rec0 = obp.tile([1, IC], F32,
                                            name=f"rec_{p}_{h}_{rep}", tag="rec")
                            nc.vector.reciprocal(rec0[:], osb[64:65, :])
                            rb = obp.tile([64, IC], F32,
                                          name=f"rb_{p}_{h}_{rep}", tag="rb")
                            nc.gpsimd.partition_broadcast(rb[:], rec0[:])
                            ot = obp.tile([64, IC], F32,
                                          name=f"ot_{p}_{h}_{rep}", tag="ot")
                            nc.gpsimd.tensor_mul(ot[:], osb[0:64, :], rb[:])
                            nc.sync.dma_start(out_dr[2 * p + h, :, ds(ci, IC)],
                                              ot[:])

    nc.compile()
    return nc


def _prep_inputs(query, key, value, Wq, bq, Wk, bk, Wv, bv):
    """Host-side sharding/layout prep. Returns per-core input maps."""
    def d_major(x):
        # [B, S, H, D] -> [B*H, D, S]
        xt = np.asarray(x, np.float32).transpose(0, 2, 3, 1)
        return np.ascontiguousarray(xt.reshape(B * H, D, S))

    qh, kh, vh = d_major(query), d_major(key), d_major(value)

    def blockdiag(W):
        Wt = np.asarray(W, np.float32).T  # [d, e]
        W2 = np.zeros((128, 128), np.float32)
        W2[:64, :64] = Wt
        W2[64:, 64:] = Wt
        return W2

    wva = np.zeros((65, 66), np.float32)
    wva[:64, :64] = np.asarray(Wv, np.float32).T
    wva[64, :64] = np.asarray(bv, np.float32)
    wva[64, 64] = 1.0

    def bias2(b):
        return np.concatenate([np.asarray(b, np.float32)] * 2).reshape(128, 1)

    shared = dict(wq2=blockdiag(Wq), wk2=blockdiag(Wk), wva=wva,
                  bq2=bias2(bq), bk2=bias2(bk))
    in_maps = []
    for c in range(NCORES):
        sl = slice(c * HPC, (c + 1) * HPC)
        va = vh[sl].reshape(NPAIR, 2, 64, S)
        va = np.concatenate(
            [va, np.ones((NPAIR, 2, 1, S), np.float32)], axis=2)
        in_maps.append(dict(
            xqT=np.ascontiguousarray(qh[sl]).reshape(NPAIR, 128, S),
            xkT=np.ascontiguousarray(kh[sl]).reshape(NPAIR, 128, S),
            xvA=np.ascontiguousarray(va),
            **shared))
    return in_maps


def kernel(query, key, value, Wq, bq, Wk, bk, Wv, bv):
    from concourse.bass_utils import run_bass_kernel_spmd

    if "nc" not in _cache:
        _cache["nc"] = _build()
    nc = _cache["nc"]

    in_maps = _prep_inputs(query, key, value, Wq, bq, Wk, bk, Wv, bv)
    res = run_bass_kernel_spmd(nc, in_maps, list(range(NCORES)))
    out = np.stack([res.results[c]["out"] for c in range(NCORES)])  # [8,4,D,S]
    out = out.reshape(B, H, D, S).transpose(0, 3, 1, 2)  # [B,S,H,D]
    return np.ascontiguousarray(out)
